# revision 1
# baseline (speedup 1.0000x reference)
"""Trainium2 Bass kernel: 2-layer GraphSAGE (degree-normalized mean aggregation,
self-loops) + elementwise-product link-prediction MLP.

Distribution (8 NeuronCores):
  - Nodes sharded contiguously across cores (12544-row padded shards).
  - Edges sharded by RECEIVER core, sorted by (receiver block, sender chunk);
    per-core segment sums computed locally with an indicator-matmul trick
    (one-hot(edge->slot) matrices built on DVE, reduced on the PE), so no
    cross-core reduction is needed -- just an AllGather of each layer's node
    table (and of the normalized gather table).
  - Pairs sharded contiguously (pure data parallel); pair endpoints gathered
    from the AllGather'ed final table.
Gathers use the SWDGE dma_gather custom instruction (int16 indices relative to
one of 4 table chunks of <=32K rows).
"""

import os
import sys

import numpy as np

_TRN_REPO = "/opt/trn_rl_repo"
if _TRN_REPO not in sys.path:
    sys.path.insert(0, _TRN_REPO)

# ---------------------------------------------------------------- problem cfg
R = 8  # cores
D = 128  # feature dim
N = int(os.environ.get("GNN_N", 100000))

NIDX_TILES = 32  # max 128-idx tiles per dma_gather call (4096 rows = 2MB f32)
GSUP = 6  # blocks per supergroup (PSUM: one bank per block + 1 tr + 1 h)

NLOC = N // R
NB = -(-NLOC // 128)  # node blocks per core
SHARD = NB * 128
TAB = R * SHARD
NCHUNK = 4
CHUNK = TAB // NCHUNK
assert N % R == 0 and CHUNK <= 32767 and TAB % NCHUNK == 0

_TRACE = False
_LAST_EXEC_NS = None
_LAST_RESULTS = None


def _cdiv(a, b):
    return -(-a // b)


def _trow(n):
    return (n // NLOC) * SHARD + (n % NLOC)


# ---------------------------------------------------------------- host prep
def _wrap16(idx_stream):
    """int16 idx stream (len = m*128) -> [128, m*8] wrapped-16 layout."""
    m8 = len(idx_stream) // 16
    a = idx_stream.reshape(m8, 16).T  # [16, m*8]
    return np.tile(a, (8, 1)).astype(np.int16)


def _preprocess(senders, receivers, pairs):
    E = senders.shape[0]
    s = np.concatenate([senders.astype(np.int64), np.arange(N, dtype=np.int64)])
    r = np.concatenate([receivers.astype(np.int64), np.arange(N, dtype=np.int64)])

    deg = np.bincount(s, minlength=N).astype(np.float64)
    cnt = np.bincount(r, minlength=N).astype(np.float64)
    ssend_n = (1.0 / np.sqrt(np.maximum(deg, 1.0))).astype(np.float32)
    srecv_n = (np.maximum(cnt, 1.0) ** -1.5).astype(np.float32)

    def pad_shard(v):
        out = np.zeros((R, SHARD), np.float32)
        for c in range(R):
            out[c, :NLOC] = v[c * NLOC : (c + 1) * NLOC]
        return out

    ssend_sh = pad_shard(ssend_n)
    srecv_sh = pad_shard(srecv_n)

    # self-loop contributions are applied densely on-device (identity matmul),
    # so only real edges go through the gather stream
    se = senders.astype(np.int64)
    re = receivers.astype(np.int64)
    srow = _trow(se)
    rcore = re // NLOC
    rloc = re % NLOC
    eblk = rloc // 128
    eslot = rloc % 128
    ech = srow // CHUNK
    erel = (srow % CHUNK).astype(np.int64)

    # supergroups of blocks
    groups = [list(range(g, min(g + GSUP, NB))) for g in range(0, NB, GSUP)]
    sgid = np.zeros(NB, np.int64)
    for gi, g in enumerate(groups):
        for b in g:
            sgid[b] = gi

    nbuck = NB * NCHUNK
    key = eblk * NCHUNK + ech
    counts = np.zeros((R, nbuck), np.int64)
    for c in range(R):
        counts[c] = np.bincount(key[rcore == c], minlength=nbuck)
    tiles_bc = _cdiv(counts.max(axis=0), 128).reshape(NB, NCHUNK)

    # static call plan -------------------------------------------------------
    # stream order: for sg: for chunk: for b in sg: bucket(b, chunk)
    layer_calls = []  # dicts: sg, chunk, ntiles, tile_blocks, gcol, scol
    blk_total = {b: int(tiles_bc[b].sum()) for b in range(NB)}
    tcursor = 0
    for gi, g in enumerate(groups):
        for c in range(NCHUNK):
            tile_blocks = []
            for b in g:
                tile_blocks += [b] * int(tiles_bc[b, c])
            pos = 0
            while pos < len(tile_blocks):
                m = min(NIDX_TILES, len(tile_blocks) - pos)
                layer_calls.append(
                    dict(
                        sg=gi,
                        chunk=c,
                        ntiles=m,
                        tile_blocks=tile_blocks[pos : pos + m],
                        tcol=tcursor,
                    )
                )
                tcursor += m
                pos += m
    GT = tcursor  # total tiles per layer stream

    # per-core edge streams --------------------------------------------------
    gidx = np.zeros((R, 128, GT * 8), np.int16)
    gseg = np.full((R, 128, GT), -1.0, np.float32)
    # bucket edges per core
    order_key = sgid[eblk] * (NCHUNK * (NB + 1)) + ech * (NB + 1) + eblk
    for c in range(R):
        m = rcore == c
        ords = np.lexsort((np.zeros(m.sum()), order_key[m]))
        ce_rel = erel[m][ords]
        ce_slot = eslot[m][ords]
        ce_key = key[m][ords]
        # bucket boundaries in sorted order
        cnts = np.bincount(ce_key, minlength=nbuck)
        # traversal order of buckets must match stream order
        starts = {}
        off = 0
        border = []
        for gi, g in enumerate(groups):
            for ch in range(NCHUNK):
                for b in g:
                    border.append((b, ch))
        # compute per-bucket start offsets in the SORTED edge array:
        # sorted by order_key which equals traversal order
        sort_off = 0
        idx_stream = np.zeros(GT * 128, np.int64)
        seg_stream = np.full(GT * 128, -1.0, np.float32)
        out_off = 0
        for (b, ch) in border:
            k = b * NCHUNK + ch
            n = int(cnts[k])
            nt = int(tiles_bc[b, ch])
            idx_stream[out_off : out_off + n] = ce_rel[sort_off : sort_off + n]
            seg_stream[out_off : out_off + n] = ce_slot[sort_off : sort_off + n]
            sort_off += n
            out_off += nt * 128
        assert out_off == GT * 128 and sort_off == m.sum()
        gidx[c] = _wrap16(idx_stream.astype(np.int16))
        gseg[c] = seg_stream.reshape(GT, 128).T

    # pairs ------------------------------------------------------------------
    P = pairs.shape[0]
    assert P % R == 0
    PLOC = P // R
    pa = _trow(pairs[:, 0].astype(np.int64))
    pb = _trow(pairs[:, 1].astype(np.int64))
    pg = (pa // CHUNK) * NCHUNK + (pb // CHUNK)
    pcore = np.arange(P) // PLOC
    pcounts = np.zeros((R, NCHUNK * NCHUNK), np.int64)
    for c in range(R):
        pcounts[c] = np.bincount(pg[pcore == c], minlength=NCHUNK * NCHUNK)
    ptiles_g = _cdiv(pcounts.max(axis=0), 128)

    pair_calls = []  # dicts: ca, cb, ntiles, tcol
    ptcur = 0
    for g in range(NCHUNK * NCHUNK):
        nt = int(ptiles_g[g])
        pos = 0
        while pos < nt:
            mm = min(NIDX_TILES, nt - pos)
            pair_calls.append(dict(ca=g // NCHUNK, cb=g % NCHUNK, ntiles=mm, tcol=ptcur))
            ptcur += mm
            pos += mm
    PT = ptcur

    paidx = np.zeros((R, 128, PT * 8), np.int16)
    pbidx = np.zeros((R, 128, PT * 8), np.int16)
    posmap = np.full((R, PT * 128), -1, np.int64)
    for c in range(R):
        mask = pcore == c
        ids = np.nonzero(mask)[0]
        a_stream = np.zeros(PT * 128, np.int64)
        b_stream = np.zeros(PT * 128, np.int64)
        off = 0
        for g in range(NCHUNK * NCHUNK):
            nt = int(ptiles_g[g])
            sel = ids[pg[ids] == g]
            n = len(sel)
            a_stream[off : off + n] = pa[sel] % CHUNK
            b_stream[off : off + n] = pb[sel] % CHUNK
            posmap[c, off : off + n] = sel
            off += nt * 128
        paidx[c] = _wrap16(a_stream.astype(np.int16))
        pbidx[c] = _wrap16(b_stream.astype(np.int16))

    meta = dict(
        groups=groups,
        tiles_bc=tiles_bc,
        blk_total=blk_total,
        layer_calls=layer_calls,
        GT=GT,
        pair_calls=pair_calls,
        PT=PT,
    )
    data = dict(
        ssend_sh=ssend_sh,
        srecv_sh=srecv_sh,
        gidx=gidx,
        gseg=gseg,
        paidx=paidx,
        pbidx=pbidx,
        posmap=posmap,
    )
    return meta, data


# ---------------------------------------------------------------- bass build
def _build(meta, bb_val):
    from concourse import bass, mybir, bacc
    import concourse.tile as tile
    from concourse.masks import make_identity

    f32 = mybir.dt.float32
    bf16 = mybir.dt.bfloat16
    i16 = mybir.dt.int16
    GT = meta["GT"]
    PT = meta["PT"]
    groups = meta["groups"]
    tiles_bc = meta["tiles_bc"]
    blk_total = meta["blk_total"]

    nc = bacc.Bacc(
        "TRN2",
        target_bir_lowering=False,
        debug=False,
        num_devices=R,
        num_swdge_queues=4,
    )

    emb_s = nc.dram_tensor("emb_s", [SHARD, D], f32, kind="ExternalInput")
    ssend_t = nc.dram_tensor("ssend", [SHARD], f32, kind="ExternalInput")
    srecv_t = nc.dram_tensor("srecv", [SHARD], f32, kind="ExternalInput")
    gidx_t = nc.dram_tensor("gidx", [128, GT * 8], i16, kind="ExternalInput")
    gseg_t = nc.dram_tensor("gseg", [128, GT], f32, kind="ExternalInput")
    paidx_t = nc.dram_tensor("paidx", [128, PT * 8], i16, kind="ExternalInput")
    pbidx_t = nc.dram_tensor("pbidx", [128, PT * 8], i16, kind="ExternalInput")
    w1t_t = nc.dram_tensor("w1t", [D, D], f32, kind="ExternalInput")
    w1b_t = nc.dram_tensor("w1b", [D, D], f32, kind="ExternalInput")
    w2t_t = nc.dram_tensor("w2t", [D, D], f32, kind="ExternalInput")
    w2b_t = nc.dram_tensor("w2b", [D, D], f32, kind="ExternalInput")
    wa_t = nc.dram_tensor("wa", [D, D], f32, kind="ExternalInput")
    wb_t = nc.dram_tensor("wb", [D, 1], f32, kind="ExternalInput")
    b1_t = nc.dram_tensor("b1", [1, D], f32, kind="ExternalInput")
    b2_t = nc.dram_tensor("b2", [1, D], f32, kind="ExternalInput")
    ba_t = nc.dram_tensor("ba", [D, 1], f32, kind="ExternalInput")
    iota_in = nc.dram_tensor("iota", [128, 128], f32, kind="ExternalInput")
    out_t = nc.dram_tensor("scores", [PT * 128], f32, kind="ExternalOutput")

    rg = [list(range(R))]
    eq = mybir.AluOpType.is_equal
    amax = mybir.AluOpType.max
    amul = mybir.AluOpType.mult
    aadd = mybir.AluOpType.add

    def g3(ap, m):
        return ap.rearrange("p (t d) -> p t d", d=128)

    with tile.TileContext(nc) as tc:
        with (
            tc.tile_pool(name="const", bufs=1) as cp,
            tc.tile_pool(name="dram", bufs=1, space="DRAM") as dp,
        ):
            w1t = cp.tile([D, D], f32)
            nc.sync.dma_start(w1t[:, :], w1t_t[:, :])
            w1b = cp.tile([D, D], f32)
            nc.sync.dma_start(w1b[:, :], w1b_t[:, :])
            w2t = cp.tile([D, D], f32)
            nc.sync.dma_start(w2t[:, :], w2t_t[:, :])
            w2b = cp.tile([D, D], f32)
            nc.sync.dma_start(w2b[:, :], w2b_t[:, :])
            wa = cp.tile([D, D], f32)
            nc.sync.dma_start(wa[:, :], wa_t[:, :])
            wb = cp.tile([D, 1], f32)
            nc.sync.dma_start(wb[:, :], wb_t[:, :])
            b1 = cp.tile([1, D], f32)
            nc.sync.dma_start(b1[:, :], b1_t[:, :])
            b2 = cp.tile([1, D], f32)
            nc.sync.dma_start(b2[:, :], b2_t[:, :])
            ba = cp.tile([D, 1], f32)
            nc.sync.dma_start(ba[:, :], ba_t[:, :])
            iota = cp.tile([128, 128], f32)
            nc.sync.dma_start(iota[:, :], iota_in[:, :])
            ones1 = cp.tile([1, 128], f32)
            nc.vector.memset(ones1[:, :], 1.0)
            ident = cp.tile([128, 128], f32)
            make_identity(nc, ident[:, :])
            identb = cp.tile([128, 128], bf16)
            nc.vector.tensor_copy(identb[:, :], ident[:, :])

            tab1 = dp.tile([TAB, D], bf16, addr_space="Shared")
            tab2 = dp.tile([TAB, D], bf16, addr_space="Shared")
            tab3 = dp.tile([TAB, D], bf16, addr_space="Shared")
            ag1in = dp.tile([SHARD, D], bf16)
            ag2in = dp.tile([SHARD, D], bf16)
            ag3in = dp.tile([SHARD, D], bf16)
            x2s = dp.tile([SHARD, D], f32)

            # ---- P1: xnorm1 = emb * ssend, shard -> AllGather
            with tc.tile_pool(name="p1", bufs=4) as p1:
                for b in range(NB):
                    xe = p1.tile([128, D], f32, tag="xe")
                    nc.sync.dma_start(xe[:, :], emb_s[b * 128 : (b + 1) * 128, :])
                    st = p1.tile([128, 1], f32, tag="st")
                    nc.sync.dma_start(
                        st[:, :],
                        ssend_t[b * 128 : (b + 1) * 128].rearrange("(p x) -> p x", x=1),
                    )
                    xn = p1.tile([128, D], bf16, tag="xn")
                    nc.vector.tensor_scalar_mul(xn[:, :], xe[:, :], st[:, :])
                    nc.sync.dma_start(ag1in[b * 128 : (b + 1) * 128, :], xn[:, :])
            nc.gpsimd.collective_compute(
                "AllGather",
                mybir.AluOpType.bypass,
                replica_groups=rg,
                ins=[ag1in[:, :].opt()],
                outs=[tab1[:, :].opt()],
            )

            # ---- layers
            gq = [0]

            def next_queue():
                q = (gq[0] // 2) % 4
                gq[0] += 1
                return q

            def emit_layer(tab, xnsrc, x_src, wtop, wbot, bias, relu, x2_out, agin_out):
                with (
                    tc.tile_pool(name="gat", bufs=3) as gp,
                    tc.tile_pool(name="ind", bufs=2) as ip,
                    tc.tile_pool(name="gmeta", bufs=4) as mp,
                    tc.tile_pool(name="epi", bufs=3) as ep,
                    tc.tile_pool(name="agg", bufs=GSUP, space="PSUM") as aggp,
                    tc.tile_pool(name="trp", bufs=1, space="PSUM") as trp,
                    tc.tile_pool(name="hp", bufs=1, space="PSUM") as hp,
                ):
                    call_i = 0
                    for gi, g in enumerate(groups):
                        aggt = [
                            aggp.tile([128, 128], f32, tag="aggt", name=f"aggt{k}")
                            for k in range(len(g))
                        ]
                        done = {b: 0 for b in g}
                        for j, b in enumerate(g):
                            xnb = ep.tile([128, D], bf16, tag="xnb")
                            nc.sync.dma_start(xnb[:, :], xnsrc[b * 128 : (b + 1) * 128, :])
                            nc.tensor.matmul(
                                aggt[j][:, :],
                                lhsT=identb[:, :],
                                rhs=xnb[:, :],
                                start=True,
                                stop=(blk_total[b] == 0),
                            )
                        # all calls of this supergroup, in chunk order
                        while call_i < len(meta["layer_calls"]) and meta["layer_calls"][call_i]["sg"] == gi:
                            call = meta["layer_calls"][call_i]
                            call_i += 1
                            m = call["ntiles"]
                            c = call["chunk"]
                            t0 = call["tcol"]
                            idx = mp.tile([128, m * 8], i16, tag="idx")
                            nc.sync.dma_start(idx[:, :], gidx_t[:, t0 * 8 : (t0 + m) * 8])
                            seg = mp.tile([128, m], f32, tag="seg")
                            nc.sync.dma_start(seg[:, :], gseg_t[:, t0 : t0 + m])
                            gat = gp.tile([128, m * 128], bf16, tag="gat")
                            nc.gpsimd.dma_gather(
                                g3(gat[:, :], m),
                                tab[c * CHUNK : (c + 1) * CHUNK, :],
                                idx[:, :],
                                m * 128,
                                m * 128,
                                D,
                                single_packet=False,
                                queue_num=next_queue(),
                            )
                            ind = ip.tile([128, m * 128], bf16, tag="ind")
                            nc.vector.tensor_tensor(
                                out=g3(ind[:, :], m),
                                in0=seg[:, :].to_broadcast([128, m, 128]),
                                in1=iota[:, :]
                                .rearrange("p (t d) -> p t d", t=1)
                                .to_broadcast([128, m, 128]),
                                op=eq,
                            )
                            for tpos, b in enumerate(call["tile_blocks"]):
                                done[b] += 1
                                last = done[b] == blk_total[b]
                                j = g.index(b)
                                nc.tensor.matmul(
                                    aggt[j][:, :],
                                    lhsT=ind[:, tpos * 128 : (tpos + 1) * 128],
                                    rhs=gat[:, tpos * 128 : (tpos + 1) * 128],
                                    start=False,
                                    stop=last,
                                )
                        # epilogue per block
                        for j, b in enumerate(g):
                            srv = mp.tile([128, 1], f32, tag="srv")
                            nc.sync.dma_start(
                                srv[:, :],
                                srecv_t[b * 128 : (b + 1) * 128].rearrange(
                                    "(p x) -> p x", x=1
                                ),
                            )
                            xupd = ep.tile([128, D], f32, tag="xupd")
                            nc.vector.tensor_scalar_mul(xupd[:, :], aggt[j][:, :], srv[:, :])
                            ps1 = trp.tile([128, 128], f32, tag="tr")
                            nc.tensor.transpose(ps1[:, :], xupd[:, :], ident[:, :])
                            xupdT = ep.tile([128, D], f32, tag="xupdT")
                            nc.vector.tensor_copy(xupdT[:, :], ps1[:, :])
                            xe = ep.tile([128, D], f32, tag="xe2")
                            nc.sync.dma_start(xe[:, :], x_src[b * 128 : (b + 1) * 128, :])
                            ps2 = trp.tile([128, 128], f32, tag="tr")
                            nc.tensor.transpose(ps2[:, :], xe[:, :], ident[:, :])
                            xT = ep.tile([128, D], f32, tag="xT")
                            nc.vector.tensor_copy(xT[:, :], ps2[:, :])
                            hps = hp.tile([128, 128], f32, tag="h")
                            nc.tensor.matmul(hps[:, :], lhsT=xT[:, :], rhs=wtop[:, :], start=True, stop=False)
                            nc.tensor.matmul(hps[:, :], lhsT=xupdT[:, :], rhs=wbot[:, :], start=False, stop=False)
                            nc.tensor.matmul(hps[:, :], lhsT=ones1[:, :], rhs=bias[:, :], start=False, stop=True)
                            if relu:
                                hx = ep.tile([128, D], f32, tag="hx")
                                nc.vector.tensor_scalar_max(hx[:, :], hps[:, :], 0.0)
                                nc.sync.dma_start(x2_out[b * 128 : (b + 1) * 128, :], hx[:, :])
                                ssd = mp.tile([128, 1], f32, tag="ssd")
                                nc.sync.dma_start(
                                    ssd[:, :],
                                    ssend_t[b * 128 : (b + 1) * 128].rearrange(
                                        "(p x) -> p x", x=1
                                    ),
                                )
                                xn2 = ep.tile([128, D], bf16, tag="xn2")
                                nc.vector.tensor_scalar(
                                    xn2[:, :],
                                    hps[:, :],
                                    0.0,
                                    ssd[:, :],
                                    op0=amax,
                                    op1=amul,
                                )
                                nc.sync.dma_start(agin_out[b * 128 : (b + 1) * 128, :], xn2[:, :])
                            else:
                                hx = ep.tile([128, D], bf16, tag="hxb")
                                nc.vector.tensor_copy(hx[:, :], hps[:, :])
                                nc.sync.dma_start(agin_out[b * 128 : (b + 1) * 128, :], hx[:, :])

            emit_layer(tab1, ag1in, emb_s, w1t, w1b, b1, True, x2s, ag2in)
            nc.gpsimd.collective_compute(
                "AllGather",
                mybir.AluOpType.bypass,
                replica_groups=rg,
                ins=[ag2in[:, :].opt()],
                outs=[tab2[:, :].opt()],
            )
            emit_layer(tab2, ag2in, x2s, w2t, w2b, b2, False, None, ag3in)
            nc.gpsimd.collective_compute(
                "AllGather",
                mybir.AluOpType.bypass,
                replica_groups=rg,
                ins=[ag3in[:, :].opt()],
                outs=[tab3[:, :].opt()],
            )

            # ---- pairs
            with (
                tc.tile_pool(name="pgat", bufs=3) as gp,
                tc.tile_pool(name="pz", bufs=2) as zp,
                tc.tile_pool(name="pmeta", bufs=4) as mp,
                tc.tile_pool(name="pepi", bufs=3) as ep,
                tc.tile_pool(name="pzt", bufs=2, space="PSUM") as ztp,
                tc.tile_pool(name="pza", bufs=2, space="PSUM") as zap,
                tc.tile_pool(name="psc", bufs=2, space="PSUM") as scp,
            ):
                for pci, call in enumerate(meta["pair_calls"]):
                    m = call["ntiles"]
                    t0 = call["tcol"]
                    ia = mp.tile([128, m * 8], i16, tag="pia")
                    nc.sync.dma_start(ia[:, :], paidx_t[:, t0 * 8 : (t0 + m) * 8])
                    ib = mp.tile([128, m * 8], i16, tag="pib")
                    nc.sync.dma_start(ib[:, :], pbidx_t[:, t0 * 8 : (t0 + m) * 8])
                    ga = gp.tile([128, m * 128], bf16, tag="ga")
                    nc.gpsimd.dma_gather(
                        g3(ga[:, :], m),
                        tab3[call["ca"] * CHUNK : (call["ca"] + 1) * CHUNK, :],
                        ia[:, :],
                        m * 128,
                        m * 128,
                        D,
                        single_packet=False,
                        queue_num=next_queue(),
                    )
                    gb = gp.tile([128, m * 128], bf16, tag="gb")
                    nc.gpsimd.dma_gather(
                        g3(gb[:, :], m),
                        tab3[call["cb"] * CHUNK : (call["cb"] + 1) * CHUNK, :],
                        ib[:, :],
                        m * 128,
                        m * 128,
                        D,
                        single_packet=False,
                        queue_num=next_queue(),
                    )
                    z = zp.tile([128, m * 128], f32, tag="z")
                    nc.vector.tensor_mul(z[:, :], ga[:, :], gb[:, :])
                    for bt in range(0, m, 4):
                        nb = min(4, m - bt)
                        zt_ps = ztp.tile([128, 512], f32, tag="zt")
                        for i in range(nb):
                            nc.tensor.matmul(
                                zt_ps[:, i * 128 : (i + 1) * 128],
                                lhsT=z[:, (bt + i) * 128 : (bt + i + 1) * 128],
                                rhs=ident[:, :],
                                is_transpose=True,
                                start=(i == 0),
                                stop=(i == nb - 1),
                            )
                        zt = ep.tile([128, 512], f32, tag="zts")
                        nc.vector.tensor_copy(zt[:, : nb * 128], zt_ps[:, : nb * 128])
                        za_ps = zap.tile([128, 512], f32, tag="za")
                        for i in range(nb):
                            nc.tensor.matmul(
                                za_ps[:, i * 128 : (i + 1) * 128],
                                lhsT=wa[:, :],
                                rhs=zt[:, i * 128 : (i + 1) * 128],
                                start=(i == 0),
                                stop=(i == nb - 1),
                            )
                        za = ep.tile([128, 512], f32, tag="zas")
                        nc.vector.tensor_scalar(
                            za[:, : nb * 128],
                            za_ps[:, : nb * 128],
                            ba[:, :],
                            0.0,
                            op0=aadd,
                            op1=amax,
                        )
                        sc_ps = scp.tile([1, 512], f32, tag="sc")
                        for i in range(nb):
                            nc.tensor.matmul(
                                sc_ps[:, i * 128 : (i + 1) * 128],
                                lhsT=wb[:, :],
                                rhs=za[:, i * 128 : (i + 1) * 128],
                                start=(i == 0),
                                stop=(i == nb - 1),
                            )
                        sc = ep.tile([1, 512], f32, tag="scs")
                        nc.vector.tensor_scalar_add(
                            sc[:, : nb * 128], sc_ps[:, : nb * 128], float(bb_val)
                        )
                        o0 = (t0 + bt) * 128
                        nc.sync.dma_start(
                            out_t[o0 : o0 + nb * 128].rearrange("(x n) -> x n", x=1),
                            sc[:, : nb * 128],
                        )
    nc.compile()
    return nc


# ---------------------------------------------------------------- entry point
def kernel(
    node_ids,
    senders,
    receivers,
    pairs,
    emb,
    W1,
    b1,
    W2,
    b2,
    Wa,
    ba,
    Wb,
    bb,
):
    global _LAST_EXEC_NS, _LAST_RESULTS
    from concourse import bass_utils

    node_ids = np.asarray(node_ids)
    senders = np.asarray(senders).astype(np.int64)
    receivers = np.asarray(receivers).astype(np.int64)
    pairs_np = np.asarray(pairs).astype(np.int64)
    emb = np.asarray(emb, dtype=np.float32)
    W1 = np.asarray(W1, dtype=np.float32)
    b1 = np.asarray(b1, dtype=np.float32)
    W2 = np.asarray(W2, dtype=np.float32)
    b2 = np.asarray(b2, dtype=np.float32)
    Wa = np.asarray(Wa, dtype=np.float32)
    ba = np.asarray(ba, dtype=np.float32)
    Wb = np.asarray(Wb, dtype=np.float32)
    bb = np.asarray(bb, dtype=np.float32)

    # x = emb[node_ids]
    x0 = emb[np.asarray(node_ids).astype(np.int64)]

    meta, data = _preprocess(senders, receivers, pairs_np)
    nc = _build(meta, float(bb.reshape(-1)[0]))

    iota = np.tile(np.arange(128, dtype=np.float32), (128, 1))
    in_maps = []
    for c in range(R):
        emb_sh = np.zeros((SHARD, D), np.float32)
        emb_sh[:NLOC] = x0[c * NLOC : (c + 1) * NLOC]
        in_maps.append(
            dict(
                emb_s=emb_sh,
                ssend=data["ssend_sh"][c],
                srecv=data["srecv_sh"][c],
                gidx=data["gidx"][c],
                gseg=data["gseg"][c],
                paidx=data["paidx"][c],
                pbidx=data["pbidx"][c],
                w1t=np.ascontiguousarray(W1[:D]),
                w1b=np.ascontiguousarray(W1[D:]),
                w2t=np.ascontiguousarray(W2[:D]),
                w2b=np.ascontiguousarray(W2[D:]),
                wa=Wa,
                wb=Wb,
                b1=b1.reshape(1, D),
                b2=b2.reshape(1, D),
                ba=ba.reshape(D, 1),
                iota=iota,
            )
        )

    res = bass_utils.run_bass_kernel_spmd(
        nc, in_maps, core_ids=list(range(R)), trace=_TRACE
    )
    _LAST_EXEC_NS = res.exec_time_ns
    _LAST_RESULTS = res

    P = pairs_np.shape[0]
    scores = np.zeros(P, np.float32)
    for c in range(R):
        v = np.asarray(res.results[c]["scores"])
        pm = data["posmap"][c]
        mvalid = pm >= 0
        scores[pm[mvalid]] = v[mvalid]
    return scores

